# revision 1
# baseline (speedup 1.0000x reference)
"""GCN+GAT message-passing network on 8 Trainium2 NeuronCores.

Strategy (edge-parallel, dst-sharded):
  - 50000 nodes split into 8 contiguous shards (one per core).
  - Each core owns all edges whose dst lies in its shard, sorted by dst into
    windows of 128 dst nodes; windows processed M_WIN at a time (a "group").
  - Per layer, a per-node table T (projected features, pre-scaled, with
    attention dot-products and a ones column appended) lives in DRAM on every
    core; per-edge source rows are fetched with gpsimd.dma_gather (int16
    indices; rows >= 32768 go through a second call against an offset view).
  - Per-dst segment sums are one-hot matmuls: lhsT = (iota == dst_slot) built
    on DVE, accumulated in PSUM per 128-dst window.
  - Edge softmax: exp(leaky(a_s[src]+a_d[dst]) - m) with a safe global upper
    bound m = max(a_s) + max(a_d); normalization via the aggregated ones
    column.  Self loops are applied analytically per window (no gather).
  - Tables are rebuilt each layer from the aggregated output shard and
    AllGather'ed across the 8 cores.
"""

import sys

sys.path.insert(0, "/opt/trn_rl_repo")

import numpy as np

import concourse.bass as bass
import concourse.bacc as bacc
import concourse.bass_isa as bass_isa
import concourse.tile as tile
from concourse import mybir
from concourse.bass_utils import run_bass_kernel_spmd
from concourse.library_config import mlp
from concourse.tile_rust import add_dep_helper

F32 = mybir.dt.float32
I16 = mybir.dt.int16
AF = mybir.ActivationFunctionType
OP = mybir.AluOpType

P = 128
NCORES = 8
M_WIN = 3          # windows per gather group
HALF = 32768       # int16 index limit; rows >= HALF go through call B
IN_C, HID, OUT_C, HEADS = 128, 64, 32, 2
NEG_SLOPE = 0.2
DBG_NO_MAXRED = False
DBG_MAX_LAYER = 4
DBG_NO_GATHER = False
DBG_NO_SHARED = False

# per-layer gather-table row sizes (f32 elements; rows must be 256B multiples)
ELEMS = {1: 64, 2: 64, 3: 192, 4: 128}
# valid leading columns of each row: [h0|1|h1|1|as0|as1] for GAT layers
ROWW = {1: 64, 2: 64, 3: 132, 4: 68}
CDIM = {1: HID, 2: HID, 3: HID, 4: OUT_C}


class Group:
    __slots__ = ("slot0", "LA", "LB", "K", "windows", "chunks", "pairs")


# --------------------------------------------------------------------------
# host-side schedule construction
# --------------------------------------------------------------------------

def build_schedule(edge_index, N):
    n_sh = N // NCORES
    W = -(-n_sh // P)
    n_groups = -(-W // M_WIN)

    src = np.asarray(edge_index[0], dtype=np.int64)
    dst = np.asarray(edge_index[1], dtype=np.int64)
    deg = np.bincount(dst, minlength=N).astype(np.float64) + 1.0
    dinv = (1.0 / np.sqrt(deg)).astype(np.float32)

    cores = []
    for c in range(NCORES):
        sel = (dst >= c * n_sh) & (dst < (c + 1) * n_sh)
        s_c = src[sel]
        d_c = dst[sel] - c * n_sh
        w_c = d_c // P
        g_c = w_c // M_WIN
        half = (s_c >= HALF).astype(np.int64)
        order = np.lexsort((d_c, w_c, half, g_c))
        cores.append((s_c[order], d_c[order], w_c[order], g_c[order], half[order]))

    cnt = np.zeros((NCORES, n_groups, 2), dtype=np.int64)
    for c in range(NCORES):
        _, _, _, g_c, half = cores[c]
        np.add.at(cnt[c], (g_c, half), 1)

    groups = []
    slot0 = 0
    for g in range(n_groups):
        gr = Group()
        gr.slot0 = slot0
        gr.LA = int(-(-cnt[:, g, 0].max() // P) * P)
        gr.LB = int(-(-cnt[:, g, 1].max() // P) * P)
        gr.K = (gr.LA + gr.LB) // P
        gr.windows = list(range(g * M_WIN, min((g + 1) * M_WIN, W)))
        gr.chunks = [[] for _ in range(gr.K)]
        groups.append(gr)
        slot0 += gr.LA + gr.LB
    S_total = slot0

    # per-core slot index per (ordered) edge
    slot_of = []
    for c in range(NCORES):
        s_c, d_c, w_c, g_c, half = cores[c]
        slots = np.empty(len(s_c), dtype=np.int64)
        pos = 0
        for g in range(n_groups):
            gr = groups[g]
            nA = int(cnt[c, g, 0])
            nB = int(cnt[c, g, 1])
            slots[pos:pos + nA] = gr.slot0 + np.arange(nA)
            slots[pos + nA:pos + nA + nB] = gr.slot0 + gr.LA + np.arange(nB)
            pos += nA + nB
        slot_of.append(slots)

    # union (chunk, window) pair structure
    pair_keys = set()
    for c in range(NCORES):
        _, _, w_c, g_c, _ = cores[c]
        slots = slot_of[c]
        gr0 = np.array([groups[g].slot0 for g in g_c])
        k_loc = (slots - gr0) // P
        w_loc = w_c - g_c * M_WIN
        pair_keys.update(zip(g_c.tolist(), k_loc.tolist(), w_loc.tolist()))
    pair_cols = {}
    for i, key in enumerate(sorted(pair_keys)):
        g, k, w = key
        pair_cols[key] = i
        groups[g].chunks[k].append((w, i))
    n_pairs = len(pair_cols)
    for gr in groups:
        for lst in gr.chunks:
            lst.sort()
    for g, gr in enumerate(groups):
        gr.pairs = {}
        for wl in range(len(gr.windows)):
            gr.pairs[wl] = [(k, col) for k in range(gr.K)
                            for (wx, col) in gr.chunks[k] if wx == wl]

    # per-core arrays
    idx_all, dstv_all = [], []
    for c in range(NCORES):
        s_c, d_c, w_c, g_c, half = cores[c]
        slots = slot_of[c]
        idx_flat = np.zeros(S_total, dtype=np.int16)
        idx_flat[slots] = np.where(half == 0, s_c, s_c - HALF).astype(np.int16)
        idx_w = idx_flat.reshape(S_total // 16, 16).T
        idx_all.append(np.tile(idx_w, (8, 1)).copy())

        dstv = np.full((P, max(n_pairs, 1)), -1.0, dtype=np.float32)
        gr0 = np.array([groups[g].slot0 for g in g_c])
        k_loc = (slots - gr0) // P
        w_loc = w_c - g_c * M_WIN
        cols = np.array([pair_cols[key] for key in
                         zip(g_c.tolist(), k_loc.tolist(), w_loc.tolist())])
        dstv[slots % P, cols] = (d_c - w_c * P).astype(np.float32)
        dstv_all.append(dstv)

    return (dict(N=N, n_sh=n_sh, W=W, groups=groups, n_pairs=n_pairs,
                 S_total=S_total, dinv=dinv),
            idx_all, dstv_all)


# --------------------------------------------------------------------------
# bass kernel builder
# --------------------------------------------------------------------------

def build_nc(sched):
    N = sched["N"]
    n_sh = sched["n_sh"]
    W = sched["W"]
    groups = sched["groups"]
    n_pairs = sched["n_pairs"]
    S_total = sched["S_total"]
    wlast = n_sh - (W - 1) * P

    def wlen(w):
        return wlast if w == W - 1 else P

    # SWDGE ring rows are 16-wide vector pushes: a call consumes
    # nidx*packets_per_idx/16 rows of the scratch/16-row ring. Keep calls
    # under capacity (biggest call ~4.6K idx x 3 packets = 864 rows < 1024).
    for gr in groups:
        for L in (1, 2, 3, 4):
            dpi = (ELEMS[L] * 4 + 16383) // 16384
            assert (max(gr.LA, gr.LB) * dpi) // 16 + 8 < 1024, (gr.LA, gr.LB, L)
    nc = bacc.Bacc(None, target_bir_lowering=False)

    xT = nc.dram_tensor("xT", [IN_C, n_sh], F32, kind="ExternalInput")
    idxs = nc.dram_tensor("idxs", [P, S_total // 16], I16, kind="ExternalInput")
    dstv_d = nc.dram_tensor("dstv", [P, max(n_pairs, 1)], F32, kind="ExternalInput")
    dinv_d = nc.dram_tensor("dinv_col", [P, W], F32, kind="ExternalInput")
    W1_d = nc.dram_tensor("W1", [IN_C, HID], F32, kind="ExternalInput")
    W2_d = nc.dram_tensor("W2", [HID, HID], F32, kind="ExternalInput")
    Wg1_d = nc.dram_tensor("Wg1", [HID, HEADS * HID], F32, kind="ExternalInput")
    Wg2_d = nc.dram_tensor("Wg2", [HEADS * HID, HEADS * OUT_C], F32, kind="ExternalInput")
    b1_d = nc.dram_tensor("b1r", [P, HID], F32, kind="ExternalInput")
    b2_d = nc.dram_tensor("b2r", [P, HID], F32, kind="ExternalInput")
    bg1_d = nc.dram_tensor("bg1r", [P, HEADS * HID], F32, kind="ExternalInput")
    bg2_d = nc.dram_tensor("bg2r", [P, OUT_C], F32, kind="ExternalInput")
    as1_d = nc.dram_tensor("as1r", [P, HEADS * HID], F32, kind="ExternalInput")
    ad1_d = nc.dram_tensor("ad1r", [P, HEADS * HID], F32, kind="ExternalInput")
    as2_d = nc.dram_tensor("as2r", [P, HEADS * OUT_C], F32, kind="ExternalInput")
    ad2_d = nc.dram_tensor("ad2r", [P, HEADS * OUT_C], F32, kind="ExternalInput")
    iota_d = nc.dram_tensor("iota_row", [P, P], F32, kind="ExternalInput")
    pcol_d = nc.dram_tensor("pcol", [P, 1], F32, kind="ExternalInput")
    ident_d = nc.dram_tensor("ident", [P, P], F32, kind="ExternalInput")
    out_d = nc.dram_tensor("out", [n_sh, OUT_C], F32, kind="ExternalOutput")

    rg = [list(range(NCORES))]

    with tile.TileContext(nc) as tc:
        with (
            tc.tile_pool(name="const", bufs=1) as cpool,
            tc.tile_pool(name="gbuf", bufs=2) as gpool,
            tc.tile_pool(name="work", bufs=3) as wpool,
            tc.tile_pool(name="small", bufs=4) as spool,
            tc.tile_pool(name="resid", bufs=1) as rpool,
            tc.tile_pool(name="uwin", bufs=4, space="PSUM") as upool,
            tc.tile_pool(name="btr", bufs=1, space="PSUM") as btrpool,
            tc.tile_pool(name="admm", bufs=1, space="PSUM") as apool,
            tc.tile_pool(name="epi", bufs=2, space="PSUM") as epool,
            tc.tile_pool(name="dram", bufs=1, space="DRAM") as dpool,
        ):
            nc.gpsimd.load_library(mlp)

            def load_const(src, shape, dt=F32):
                nm = f"c_{src.name}"
                t = cpool.tile(shape, dt, name=nm, tag=nm)
                nc.sync.dma_start(t[:], src[:])
                return t

            idx_sb = load_const(idxs, [P, S_total // 16], I16)
            dstv_sb = load_const(dstv_d, [P, max(n_pairs, 1)])
            dinv_sb = load_const(dinv_d, [P, W])
            xT_sb = load_const(xT, [IN_C, n_sh])
            W1_sb = load_const(W1_d, [IN_C, HID])
            W2_sb = load_const(W2_d, [HID, HID])
            Wg1_sb = load_const(Wg1_d, [HID, HEADS * HID])
            Wg2_sb = load_const(Wg2_d, [HEADS * HID, HEADS * OUT_C])
            b1_sb = load_const(b1_d, [P, HID])
            b2_sb = load_const(b2_d, [P, HID])
            bg1_sb = load_const(bg1_d, [P, HEADS * HID])
            bg2_sb = load_const(bg2_d, [P, OUT_C])
            att_sb = {3: (load_const(as1_d, [P, HEADS * HID]),
                          load_const(ad1_d, [P, HEADS * HID])),
                      4: (load_const(as2_d, [P, HEADS * OUT_C]),
                          load_const(ad2_d, [P, HEADS * OUT_C]))}
            iota_sb = load_const(iota_d, [P, P])
            pcol_sb = load_const(pcol_d, [P, 1])
            ident_sb = load_const(ident_d, [P, P])

            # per-GAT-layer residents
            ad_sh = {L: rpool.tile([P, 2 * W], F32, tag=f"adsh{L}", name=f"adsh{L}") for L in (3, 4)}
            asmax = {L: rpool.tile([P, 2], F32, tag=f"asmax{L}", name=f"asmax{L}") for L in (3, 4)}
            admax = {L: rpool.tile([P, 2], F32, tag=f"admax{L}", name=f"admax{L}") for L in (3, 4)}
            negm = {L: rpool.tile([P, 2], F32, tag=f"negm{L}", name=f"negm{L}") for L in (3, 4)}

            ag_in = {L: dpool.tile([n_sh, ELEMS[L]], F32, tag=f"agin{L}", name=f"agin{L}")
                     for L in (1, 2, 3, 4)}
            T = {L: dpool.tile([N, ELEMS[L]], F32, tag=f"T{L}", name=f"Tbl{L}",
                               addr_space=("Local" if DBG_NO_SHARED else "Shared"))
                 for L in (1, 2, 3, 4)}
            mx_in = {L: dpool.tile([1, 8], F32, tag=f"mxin{L}", name=f"mxin{L}") for L in (3, 4)}
            mx_out = {L: dpool.tile([1, 8], F32, tag=f"mxout{L}", name=f"mxout{L}",
                                    addr_space="Shared") for L in (3, 4)}

            for L in (3, 4):
                nc.vector.memset(asmax[L][:], -3.0e38)
                nc.vector.memset(admax[L][:], -3.0e38)
                nc.vector.memset(ad_sh[L][:], 0.0)

            # ------------- helpers -------------
            def build_gat_row(L, w, h_ps):
                """h_ps [P, HEADS*C] in PSUM -> table row for GAT layer L."""
                C = CDIM[L]
                as_sb, adr_sb = att_sb[L]
                n = wlen(w)
                row = wpool.tile([P, ELEMS[L]], F32, tag="trow")
                nc.vector.memset(row[:], 0.0)
                nc.vector.tensor_copy(row[:, 0:C], h_ps[:, 0:C])
                nc.vector.memset(row[:, C:C + 1], 1.0)
                nc.vector.tensor_copy(row[:, C + 1:2 * C + 1], h_ps[:, C:2 * C])
                nc.vector.memset(row[:, 2 * C + 1:2 * C + 2], 1.0)
                prod = wpool.tile([P, HEADS * C], F32, tag="prod")
                nc.vector.tensor_tensor(out=prod[:], in0=h_ps[:], in1=as_sb[:], op=OP.mult)
                nc.vector.tensor_reduce(
                    out=row[:, 2 * C + 2:2 * C + 4],
                    in_=prod[:].rearrange("p (h c) -> p h c", h=HEADS),
                    axis=mybir.AxisListType.X, op=OP.add)
                nc.vector.tensor_tensor(out=prod[:], in0=h_ps[:], in1=adr_sb[:], op=OP.mult)
                nc.vector.tensor_reduce(
                    out=ad_sh[L][:, 2 * w:2 * w + 2],
                    in_=prod[:].rearrange("p (h c) -> p h c", h=HEADS),
                    axis=mybir.AxisListType.X, op=OP.add)
                nc.vector.tensor_tensor(out=asmax[L][:], in0=asmax[L][:],
                                        in1=row[:, 2 * C + 2:2 * C + 4], op=OP.max)
                nc.vector.tensor_tensor(out=admax[L][:], in0=admax[L][:],
                                        in1=ad_sh[L][:, 2 * w:2 * w + 2], op=OP.max)
                nc.sync.dma_start(ag_in[L][w * P:w * P + n, :], row[:n, :])

            def finish_gcn_table(layer, w, r_sb):
                """relu'd [P, HID] output of GCN layer -> next layer's table row."""
                n = wlen(w)
                rT_ps = epool.tile([P, P], F32, tag="epi")
                nc.tensor.transpose(out=rT_ps[:HID, :], in_=r_sb[:], identity=ident_sb[:])
                rT_sb = wpool.tile([HID, P], F32, tag="rT")
                nc.vector.tensor_copy(rT_sb[:], rT_ps[:HID, :])
                if layer == 1:
                    h_ps = epool.tile([P, P], F32, tag="epi")
                    nc.tensor.matmul(h_ps[:, :HID], lhsT=rT_sb[:], rhs=W2_sb[:],
                                     start=True, stop=True)
                    t_sb = wpool.tile([P, HID], F32, tag="trow")
                    nc.vector.tensor_scalar(out=t_sb[:], in0=h_ps[:, :HID],
                                            scalar1=dinv_sb[:, w:w + 1],
                                            scalar2=None, op0=OP.mult)
                    nc.sync.dma_start(ag_in[2][w * P:w * P + n, :], t_sb[:n, :])
                else:
                    h_ps = epool.tile([P, P], F32, tag="epi")
                    nc.tensor.matmul(h_ps[:, :HEADS * HID], lhsT=rT_sb[:], rhs=Wg1_sb[:],
                                     start=True, stop=True)
                    build_gat_row(3, w, h_ps[:, :HEADS * HID])

            def epilogue(layer, w, U):
                n = wlen(w)
                C = CDIM[layer]
                roww = ROWW[layer]
                own = wpool.tile([P, roww], F32, tag="own")
                if n < P:
                    nc.vector.memset(own[:], 0.0)
                nc.sync.dma_start(own[:n, :], ag_in[layer][w * P:w * P + n, 0:roww])

                if layer <= 2:
                    t = wpool.tile([P, HID], F32, tag="t1")
                    nc.vector.tensor_tensor(out=t[:], in0=U[:], in1=own[:], op=OP.add)
                    nc.vector.tensor_scalar(out=t[:], in0=t[:],
                                            scalar1=dinv_sb[:, w:w + 1],
                                            scalar2=None, op0=OP.mult)
                    nc.vector.tensor_tensor(out=t[:], in0=t[:],
                                            in1=(b1_sb if layer == 1 else b2_sb)[:],
                                            op=OP.add)
                    r = wpool.tile([P, HID], F32, tag="r1")
                    nc.scalar.activation(r[:], t[:], AF.Relu)
                    finish_gcn_table(layer, w, r)
                    return

                # GAT: add self-loop term, then normalize
                es = spool.tile([P, 2], F32, tag="es")
                nc.vector.tensor_tensor(out=es[:], in0=own[:, 2 * C + 2:2 * C + 4],
                                        in1=ad_sh[layer][:, 2 * w:2 * w + 2], op=OP.add)
                est = spool.tile([P, 2], F32, tag="est")
                nc.vector.tensor_scalar(out=est[:], in0=es[:], scalar1=NEG_SLOPE,
                                        scalar2=None, op0=OP.mult)
                nc.vector.tensor_tensor(out=es[:], in0=es[:], in1=est[:], op=OP.max)
                exs = spool.tile([P, 2], F32, tag="exs")
                for h in range(2):
                    nc.scalar.activation(exs[:, h:h + 1], es[:, h:h + 1], AF.Exp,
                                         bias=negm[layer][:, h:h + 1], scale=1.0)
                stg = wpool.tile([P, 2 * (C + 1)], F32, tag="stgs")
                for h in range(2):
                    nc.vector.tensor_scalar(
                        out=stg[:, h * (C + 1):(h + 1) * (C + 1)],
                        in0=own[:, h * (C + 1):(h + 1) * (C + 1)],
                        scalar1=exs[:, h:h + 1], scalar2=None, op0=OP.mult)
                nc.vector.tensor_tensor(out=U[:], in0=U[:], in1=stg[:], op=OP.add)

                sden = spool.tile([P, 2], F32, tag="sden")
                for h in range(2):
                    nc.vector.tensor_scalar(out=sden[:, h:h + 1],
                                            in0=U[:, h * (C + 1) + C:(h + 1) * (C + 1)],
                                            scalar1=1.0e-30, scalar2=None, op0=OP.max)
                rs = spool.tile([P, 2], F32, tag="rs")
                nc.vector.reciprocal(rs[:], sden[:])

                if layer == 3:
                    cat = wpool.tile([P, HEADS * HID], F32, tag="cat")
                    for h in range(2):
                        nc.vector.tensor_scalar(
                            out=cat[:, h * C:(h + 1) * C],
                            in0=U[:, h * (C + 1):h * (C + 1) + C],
                            scalar1=rs[:, h:h + 1], scalar2=None, op0=OP.mult)
                    nc.vector.tensor_tensor(out=cat[:], in0=cat[:], in1=bg1_sb[:], op=OP.add)
                    r = wpool.tile([P, HEADS * HID], F32, tag="cat2")
                    nc.scalar.activation(r[:], cat[:], AF.Relu)
                    rT_ps = epool.tile([P, P], F32, tag="epi")
                    nc.tensor.transpose(out=rT_ps[:], in_=r[:], identity=ident_sb[:])
                    rT_sb = wpool.tile([P, P], F32, tag="rT")
                    nc.vector.tensor_copy(rT_sb[:], rT_ps[:])
                    h_ps = epool.tile([P, P], F32, tag="epi")
                    nc.tensor.matmul(h_ps[:, :HEADS * OUT_C], lhsT=rT_sb[:], rhs=Wg2_sb[:],
                                     start=True, stop=True)
                    build_gat_row(4, w, h_ps[:, :HEADS * OUT_C])
                    return

                # layer 4: mean heads + bias + log_softmax -> output
                m0 = spool.tile([P, OUT_C], F32, tag="m0")
                nc.vector.tensor_scalar(out=m0[:], in0=U[:, 0:OUT_C],
                                        scalar1=rs[:, 0:1], scalar2=0.5,
                                        op0=OP.mult, op1=OP.mult)
                m1 = spool.tile([P, OUT_C], F32, tag="m1")
                nc.vector.tensor_scalar(out=m1[:], in0=U[:, OUT_C + 1:2 * OUT_C + 1],
                                        scalar1=rs[:, 1:2], scalar2=0.5,
                                        op0=OP.mult, op1=OP.mult)
                z = wpool.tile([P, OUT_C], F32, tag="z")
                nc.vector.tensor_tensor(out=z[:], in0=m0[:], in1=m1[:], op=OP.add)
                nc.vector.tensor_tensor(out=z[:], in0=z[:], in1=bg2_sb[:], op=OP.add)
                mx = spool.tile([P, 1], F32, tag="mx")
                nc.vector.tensor_reduce(out=mx[:], in_=z[:],
                                        axis=mybir.AxisListType.X, op=OP.max)
                nmx = spool.tile([P, 1], F32, tag="nmx")
                nc.vector.tensor_scalar(out=nmx[:], in0=mx[:], scalar1=-1.0,
                                        scalar2=None, op0=OP.mult)
                ez = wpool.tile([P, OUT_C], F32, tag="ez")
                ssum = spool.tile([P, 1], F32, tag="ssum")
                nc.scalar.activation(ez[:], z[:], AF.Exp, bias=nmx[:], scale=1.0,
                                     accum_out=ssum[:])
                lns = spool.tile([P, 1], F32, tag="lns")
                nc.scalar.activation(lns[:], ssum[:], AF.Ln)
                o = wpool.tile([P, OUT_C], F32, tag="o")
                nc.vector.tensor_scalar(out=o[:], in0=z[:], scalar1=mx[:],
                                        scalar2=lns[:], op0=OP.subtract, op1=OP.subtract)
                nc.sync.dma_start(out_d[w * P:w * P + n, :], o[:n, :])

            def edge_pass(layer):
                elem = ELEMS[layer]
                is_gat = layer >= 3
                C = CDIM[layer]
                ucols = HID if not is_gat else HEADS * (C + 1)

                if is_gat and DBG_NO_MAXRED:
                    nc.vector.memset(negm[layer][:], 0.0)
                elif is_gat:
                    nc.gpsimd.partition_all_reduce(
                        out_ap=asmax[layer][:], in_ap=asmax[layer][:], channels=P,
                        reduce_op=bass_isa.ReduceOp.max)
                    nc.gpsimd.partition_all_reduce(
                        out_ap=admax[layer][:], in_ap=admax[layer][:], channels=P,
                        reduce_op=bass_isa.ReduceOp.max)
                    mx8 = spool.tile([1, 8], F32, tag="mx8")
                    nc.vector.memset(mx8[:], -3.0e38)
                    nc.vector.tensor_copy(mx8[:, 0:2], asmax[layer][0:1, :])
                    nc.sync.dma_start(mx_in[layer][:], mx8[:])
                    nc.gpsimd.collective_compute(
                        "AllReduce", OP.max, replica_groups=rg,
                        ins=[mx_in[layer].opt()], outs=[mx_out[layer].opt()])
                    asg = spool.tile([1, 8], F32, tag="asg")
                    nc.sync.dma_start(asg[:], mx_out[layer][:])
                    asgb = spool.tile([P, 8], F32, tag="asgb")
                    nc.gpsimd.partition_broadcast(out_ap=asgb[:], in_ap=asg[:])
                    nc.vector.tensor_tensor(out=negm[layer][:], in0=asgb[:, 0:2],
                                            in1=admax[layer][:], op=OP.add)
                    nc.vector.tensor_scalar(out=negm[layer][:], in0=negm[layer][:],
                                            scalar1=-1.0, scalar2=None, op0=OP.mult)

                for g, gr in enumerate(groups):
                    K = gr.K
                    G = gpool.tile([P, K * elem], F32, tag="G")
                    kA = gr.LA // P
                    if DBG_NO_GATHER:
                        nc.sync.dma_start(
                            G[:].rearrange("p (n e) -> p n e", e=elem),
                            T[layer][0:gr.K, :][None, :, :].to_broadcast(
                                [P, gr.K, elem]) if False else
                            T[layer][0:gr.K * P, :].rearrange(
                                "(n p) e -> p n e", p=P))
                    elif gr.LA:
                        creg = nc.gpsimd.alloc_register(f"gcA_{layer}_{g}")
                        mv = nc.gpsimd.reg_mov(creg, gr.LA)
                        gi = nc.gpsimd.dma_gather(
                            G[:, :kA * elem].rearrange("p (n e) -> p n e", e=elem),
                            T[layer][:],
                            idx_sb[:, gr.slot0 // 16:(gr.slot0 + gr.LA) // 16],
                            gr.LA, creg, elem,
                            single_packet=(gr.LA <= 1024))
                        add_dep_helper(gi.ins, mv.ins, sync=False,
                                       reason="gather count reg def before use")
                    if gr.LB and not DBG_NO_GATHER:
                        creg = nc.gpsimd.alloc_register(f"gcB_{layer}_{g}")
                        mv = nc.gpsimd.reg_mov(creg, gr.LB)
                        gi = nc.gpsimd.dma_gather(
                            G[:, kA * elem:].rearrange("p (n e) -> p n e", e=elem),
                            T[layer][HALF:, :],
                            idx_sb[:, (gr.slot0 + gr.LA) // 16:
                                   (gr.slot0 + gr.LA + gr.LB) // 16],
                            gr.LB, creg, elem,
                            single_packet=(gr.LB <= 1024))
                        add_dep_helper(gi.ins, mv.ins, sync=False,
                                       reason="gather count reg def before use")

                    U = {wl: upool.tile([P, ucols], F32, tag="uwin", name=f"U{layer}_{g}_{wl}")
                         for wl in range(len(gr.windows))}
                    seen = {wl: 0 for wl in U}
                    total = {wl: len(gr.pairs[wl]) for wl in U}

                    for k in range(K):
                        plist = gr.chunks[k]
                        if not plist:
                            continue
                        if is_gat:
                            admm = apool.tile([P, 2], F32, tag="admm")
                            for pi, (wl, col) in enumerate(plist):
                                dps = btrpool.tile([P, P], F32, tag="btr")
                                nc.tensor.transpose(
                                    out=dps[:],
                                    in_=dstv_sb[:, col:col + 1].to_broadcast([P, P]),
                                    identity=ident_sb[:])
                                B = wpool.tile([P, P], F32, tag="B")
                                nc.vector.tensor_scalar(
                                    out=B[:], in0=dps[:], scalar1=pcol_sb[:],
                                    scalar2=None, op0=OP.is_equal)
                                w_abs = gr.windows[wl]
                                nc.tensor.matmul(
                                    admm[:], lhsT=B[:],
                                    rhs=ad_sh[layer][:, 2 * w_abs:2 * w_abs + 2],
                                    start=(pi == 0), stop=(pi == len(plist) - 1))
                            e_sb = spool.tile([P, 2], F32, tag="e")
                            nc.vector.tensor_tensor(
                                out=e_sb[:], in0=admm[:],
                                in1=G[:, k * elem + 2 * C + 2:k * elem + 2 * C + 4],
                                op=OP.add)
                            elk = spool.tile([P, 2], F32, tag="elk")
                            nc.vector.tensor_scalar(out=elk[:], in0=e_sb[:],
                                                    scalar1=NEG_SLOPE,
                                                    scalar2=None, op0=OP.mult)
                            nc.vector.tensor_tensor(out=e_sb[:], in0=e_sb[:],
                                                    in1=elk[:], op=OP.max)
                            ex = spool.tile([P, 2], F32, tag="ex")
                            for h in range(2):
                                nc.scalar.activation(ex[:, h:h + 1], e_sb[:, h:h + 1],
                                                     AF.Exp, bias=negm[layer][:, h:h + 1],
                                                     scale=1.0)
                            stg = wpool.tile([P, 2 * (C + 1)], F32, tag="stg")
                            for h in range(2):
                                nc.vector.tensor_scalar(
                                    out=stg[:, h * (C + 1):(h + 1) * (C + 1)],
                                    in0=G[:, k * elem + h * (C + 1):
                                         k * elem + (h + 1) * (C + 1)],
                                    scalar1=ex[:, h:h + 1], scalar2=None, op0=OP.mult)
                            rhs = stg[:]
                        else:
                            rhs = G[:, k * elem:(k + 1) * elem]

                        for (wl, col) in plist:
                            A = wpool.tile([P, P], F32, tag="A")
                            nc.vector.tensor_scalar(
                                out=A[:], in0=iota_sb[:],
                                scalar1=dstv_sb[:, col:col + 1],
                                scalar2=None, op0=OP.is_equal)
                            nc.tensor.matmul(U[wl][:], lhsT=A[:], rhs=rhs,
                                             start=(seen[wl] == 0),
                                             stop=(seen[wl] == total[wl] - 1))
                            seen[wl] += 1

                    for wl, w in enumerate(gr.windows):
                        epilogue(layer, w, U[wl])

            def allgather(L):
                nc.gpsimd.collective_compute(
                    "AllGather", OP.bypass, replica_groups=rg,
                    ins=[ag_in[L].opt()], outs=[T[L].opt()])

            # ---------------- main program ----------------
            for w in range(W):
                n = wlen(w)
                h_ps = epool.tile([P, P], F32, tag="epi")
                nc.tensor.matmul(h_ps[:n, :HID], lhsT=xT_sb[:, w * P:w * P + n],
                                 rhs=W1_sb[:], start=True, stop=True)
                t_sb = wpool.tile([P, HID], F32, tag="trow")
                nc.vector.tensor_scalar(out=t_sb[:n, :], in0=h_ps[:n, :HID],
                                        scalar1=dinv_sb[:n, w:w + 1],
                                        scalar2=None, op0=OP.mult)
                nc.sync.dma_start(ag_in[1][w * P:w * P + n, :], t_sb[:n, :])

            allgather(1)
            if DBG_MAX_LAYER >= 1:
                edge_pass(1)
            if DBG_MAX_LAYER >= 2:
                allgather(2)
                edge_pass(2)
            if DBG_MAX_LAYER >= 3:
                allgather(3)
                edge_pass(3)
            if DBG_MAX_LAYER >= 4:
                allgather(4)
                edge_pass(4)
            if DBG_MAX_LAYER < 4:
                zt = wpool.tile([P, OUT_C], F32, tag="o")
                nc.vector.memset(zt[:], 0.0)
                for w in range(W):
                    nc.sync.dma_start(out_d[w * P:w * P + wlen(w), :], zt[:wlen(w), :])

    return nc


# --------------------------------------------------------------------------
# host entry
# --------------------------------------------------------------------------

def make_inmaps(x, sched, idx_all, dstv_all, weights):
    (W1, b1, W2, b2, Wg1, as1, ad1, bg1, Wg2, as2, ad2, bg2) = weights
    n_sh = sched["n_sh"]
    Wn = sched["W"]
    dinv = sched["dinv"]

    def rep(v):
        v = np.asarray(v, np.float32).reshape(1, -1)
        return np.tile(v, (P, 1)).copy()

    iota = np.tile(np.arange(P, dtype=np.float32)[None, :], (P, 1)).copy()
    pcol = np.arange(P, dtype=np.float32)[:, None].copy()
    ident = np.eye(P, dtype=np.float32)

    in_maps = []
    for c in range(NCORES):
        flat = np.zeros(Wn * P, np.float32)
        flat[:n_sh] = dinv[c * n_sh:(c + 1) * n_sh]
        in_maps.append({
            "xT": np.ascontiguousarray(
                np.asarray(x)[c * n_sh:(c + 1) * n_sh].T).astype(np.float32),
            "idxs": idx_all[c],
            "dstv": dstv_all[c],
            "dinv_col": np.ascontiguousarray(flat.reshape(Wn, P).T),
            "W1": np.asarray(W1, np.float32), "W2": np.asarray(W2, np.float32),
            "Wg1": np.asarray(Wg1, np.float32), "Wg2": np.asarray(Wg2, np.float32),
            "b1r": rep(b1), "b2r": rep(b2), "bg1r": rep(bg1), "bg2r": rep(bg2),
            "as1r": rep(np.asarray(as1).reshape(-1)),
            "ad1r": rep(np.asarray(ad1).reshape(-1)),
            "as2r": rep(np.asarray(as2).reshape(-1)),
            "ad2r": rep(np.asarray(ad2).reshape(-1)),
            "iota_row": iota, "pcol": pcol, "ident": ident,
        })
    return in_maps


def _run_bench(inputs, n_iters=6):
    """Compile once, then time repeated executions with device-resident
    inputs (fresh donated zero-outputs each iteration). Returns
    (out, best_seconds)."""
    import time
    import jax
    from jax.sharding import Mesh, PartitionSpec, NamedSharding
    from jax.experimental.shard_map import shard_map
    from concourse import bass2jax as b2j

    x = np.asarray(inputs["x"])
    N = x.shape[0]
    sched, idx_all, dstv_all = build_schedule(np.asarray(inputs["edge_index"]), N)
    weights = tuple(inputs[k] for k in
                    ("W1", "b1", "W2", "b2", "Wg1", "as1", "ad1", "bg1",
                     "Wg2", "as2", "ad2", "bg2"))
    in_maps = make_inmaps(x, sched, idx_all, dstv_all, weights)
    nc = build_nc(sched)
    nc.compile()

    b2j.install_neuronx_cc_hook()
    partition_name = nc.partition_id_tensor.name if nc.partition_id_tensor else None
    in_names, out_names, out_avals, zero_outs = [], [], [], []
    import concourse.mybir as mb
    for alloc in nc.m.functions[0].allocations:
        if not isinstance(alloc, mb.MemoryLocationSet):
            continue
        name = alloc.memorylocations[0].name
        if alloc.kind == "ExternalInput":
            if name != partition_name:
                in_names.append(name)
        elif alloc.kind == "ExternalOutput":
            out_names.append(name)
            shape = tuple(alloc.tensor_shape)
            dtype = mb.dt.np(alloc.dtype)
            out_avals.append(jax.core.ShapedArray(shape, dtype))
            zero_outs.append(np.zeros(shape, dtype))
    n_params = len(in_names)
    n_outs = len(out_avals)
    in_names_full = in_names + out_names + ([partition_name] if partition_name else [])

    def _body(*args):
        operands = list(args)
        if partition_name is not None:
            operands.append(b2j.partition_id_tensor())
        return tuple(b2j._bass_exec_p.bind(
            *operands, out_avals=tuple(out_avals), in_names=tuple(in_names_full),
            out_names=tuple(out_names), lowering_input_output_aliases=(),
            sim_require_finite=True, sim_require_nnan=True, nc=nc))

    donate = tuple(range(n_params, n_params + n_outs))
    devices = jax.devices()[:NCORES]
    mesh = Mesh(np.asarray(devices), ("core",))
    PS = PartitionSpec("core")
    sharded = jax.jit(
        shard_map(_body, mesh=mesh, in_specs=(PS,) * (n_params + n_outs),
                  out_specs=(PS,) * n_outs, check_rep=False),
        donate_argnums=donate, keep_unused=True)
    shd = NamedSharding(mesh, PS)
    concat_in = [
        jax.device_put(np.concatenate([np.asarray(in_maps[c][nm]) for c in range(NCORES)],
                                      axis=0), shd)
        for nm in in_names]
    zshapes = [(NCORES * z.shape[0], *z.shape[1:]) for z in zero_outs]

    best = None
    outs_np = None
    for it in range(n_iters):
        zs = [jax.device_put(np.zeros(s, np.float32), shd) for s in zshapes]
        jax.block_until_ready(zs)
        t0 = time.perf_counter()
        outs = sharded(*concat_in, *zs)
        jax.block_until_ready(outs)
        dt = time.perf_counter() - t0
        print(f"  iter {it}: {dt*1e3:.3f} ms")
        if best is None or dt < best:
            best = dt
        outs_np = [np.asarray(o) for o in outs]
    i_out = out_names.index("out")
    out = outs_np[i_out].reshape(NCORES, -1, OUT_C).reshape(-1, OUT_C)
    return out.astype(np.float32), best


def _run(inputs, trace=False, tmpdir=None):
    x = np.asarray(inputs["x"])
    N = x.shape[0]
    sched, idx_all, dstv_all = build_schedule(np.asarray(inputs["edge_index"]), N)
    weights = tuple(inputs[k] for k in
                    ("W1", "b1", "W2", "b2", "Wg1", "as1", "ad1", "bg1",
                     "Wg2", "as2", "ad2", "bg2"))
    in_maps = make_inmaps(x, sched, idx_all, dstv_all, weights)
    nc = build_nc(sched)
    nc.compile()
    res = run_bass_kernel_spmd(nc, in_maps, list(range(NCORES)),
                               trace=trace, tmpdir=tmpdir)
    out = np.concatenate([res.results[c]["out"] for c in range(NCORES)], axis=0)
    return out.astype(np.float32), res


def kernel(x, edge_index, W1, b1, W2, b2, Wg1, as1, ad1, bg1, Wg2, as2, ad2, bg2):
    out, _ = _run(dict(x=x, edge_index=edge_index, W1=W1, b1=b1, W2=W2, b2=b2,
                       Wg1=Wg1, as1=as1, ad1=ad1, bg1=bg1, Wg2=Wg2, as2=as2,
                       ad2=ad2, bg2=bg2))
    return out



# revision 6
# speedup vs baseline: 1.2098x; 1.2098x over previous
"""GCN+GAT message-passing network on 8 Trainium2 NeuronCores.

Strategy (edge-parallel, dst-sharded), v2 (bf16 + batched one-hots):
  - 50000 nodes split into 8 contiguous shards (one per core).
  - Each core owns all edges whose dst lies in its shard, sorted by dst into
    windows of 128 dst nodes; windows processed M_WIN at a time (a "group").
  - Per layer, a per-node table T (projected features, pre-scaled for GCN)
    lives in DRAM on every core in bf16 with uniform 256B rows (128 bf16
    elems); per-edge source rows are fetched with gpsimd.dma_gather (int16
    indices; rows >= 32768 go through a second call against an offset view).
  - Per-dst segment sums are one-hot matmuls in bf16: all of a group's
    one-hot lhsT matrices are built in ONE batched is_equal (broadcast APs);
    the GAT "B" matrices (the transposed one-hots for the ad[dst] lookup)
    are built the same way from a host-prepared transposed dstv table.
  - GAT per-edge attention: as[src] is reconstructed on device from the
    gathered h row (one batched multiply+reduce per group), ad[dst] comes
    from a per-pair matmul with B; the whole exp pipeline runs batched per
    group ([P, 2K] tiles); softmax normalization via an exs column appended
    to the staged rhs.  Self loops are applied analytically per window.
  - Edge softmax max bound: m = allreduce_max(as) + local max(ad).
  - Tables are rebuilt each layer from the aggregated output shard and
    AllGather'ed across the 8 cores.
"""

import sys

sys.path.insert(0, "/opt/trn_rl_repo")

import numpy as np
import ml_dtypes

import concourse.bass as bass
import concourse.bacc as bacc
import concourse.bass_isa as bass_isa
import concourse.tile as tile
from concourse import mybir
from concourse.bass_utils import run_bass_kernel_spmd
from concourse.library_config import mlp
from concourse.tile_rust import add_dep_helper

F32 = mybir.dt.float32
BF16 = mybir.dt.bfloat16
I16 = mybir.dt.int16
AF = mybir.ActivationFunctionType
OP = mybir.AluOpType
BFNP = ml_dtypes.bfloat16

P = 128
NCORES = 8
M_WIN = 3          # windows per gather group
HALF = 32768       # int16 index limit; rows >= HALF go through call B
IN_C, HID, OUT_C, HEADS = 128, 64, 32, 2
NEG_SLOPE = 0.2
DBG_NO_MAXRED = False
DBG_MAX_LAYER = 4
DBG_NO_SHARED = False
DBG_LOCAL_COLL = False     # replace collectives with local DMAs (TimelineSim)

ELEM = 128                 # bf16 elems per table row (256B, the SWDGE minimum)
# valid leading columns of each row ([h] only; as/ones are reconstructed)
VAL = {1: HID, 2: HID, 3: HEADS * HID, 4: HEADS * OUT_C}
CDIM = {1: HID, 2: HID, 3: HID, 4: OUT_C}


class Group:
    __slots__ = ("slot0", "LA", "LB", "K", "windows", "chunks", "pairs",
                 "col0", "npairs")


# --------------------------------------------------------------------------
# host-side schedule construction
# --------------------------------------------------------------------------

def build_schedule(edge_index, N):
    n_sh = N // NCORES
    W = -(-n_sh // P)
    n_groups = -(-W // M_WIN)

    src = np.asarray(edge_index[0], dtype=np.int64)
    dst = np.asarray(edge_index[1], dtype=np.int64)
    deg = np.bincount(dst, minlength=N).astype(np.float64) + 1.0
    dinv = (1.0 / np.sqrt(deg)).astype(np.float32)

    cores = []
    for c in range(NCORES):
        sel = (dst >= c * n_sh) & (dst < (c + 1) * n_sh)
        s_c = src[sel]
        d_c = dst[sel] - c * n_sh
        w_c = d_c // P
        g_c = w_c // M_WIN
        half = (s_c >= HALF).astype(np.int64)
        order = np.lexsort((d_c, w_c, half, g_c))
        cores.append((s_c[order], d_c[order], w_c[order], g_c[order], half[order]))

    cnt = np.zeros((NCORES, n_groups, 2), dtype=np.int64)
    for c in range(NCORES):
        _, _, _, g_c, half = cores[c]
        np.add.at(cnt[c], (g_c, half), 1)

    groups = []
    slot0 = 0
    for g in range(n_groups):
        gr = Group()
        gr.slot0 = slot0
        gr.LA = int(-(-cnt[:, g, 0].max() // P) * P)
        gr.LB = int(-(-cnt[:, g, 1].max() // P) * P)
        gr.K = (gr.LA + gr.LB) // P
        gr.windows = list(range(g * M_WIN, min((g + 1) * M_WIN, W)))
        gr.chunks = [[] for _ in range(gr.K)]
        groups.append(gr)
        slot0 += gr.LA + gr.LB
    S_total = slot0

    # per-core slot index per (ordered) edge
    slot_of = []
    for c in range(NCORES):
        s_c, d_c, w_c, g_c, half = cores[c]
        slots = np.empty(len(s_c), dtype=np.int64)
        pos = 0
        for g in range(n_groups):
            gr = groups[g]
            nA = int(cnt[c, g, 0])
            nB = int(cnt[c, g, 1])
            slots[pos:pos + nA] = gr.slot0 + np.arange(nA)
            slots[pos + nA:pos + nA + nB] = gr.slot0 + gr.LA + np.arange(nB)
            pos += nA + nB
        slot_of.append(slots)

    # union (chunk, window) pair structure
    pair_keys = set()
    for c in range(NCORES):
        _, _, w_c, g_c, _ = cores[c]
        slots = slot_of[c]
        gr0 = np.array([groups[g].slot0 for g in g_c])
        k_loc = (slots - gr0) // P
        w_loc = w_c - g_c * M_WIN
        pair_keys.update(zip(g_c.tolist(), k_loc.tolist(), w_loc.tolist()))
    pair_cols = {}
    for i, key in enumerate(sorted(pair_keys)):
        g, k, w = key
        pair_cols[key] = i
        groups[g].chunks[k].append((w, i))
    n_pairs = len(pair_cols)
    for gr in groups:
        for lst in gr.chunks:
            lst.sort()
    for g, gr in enumerate(groups):
        cols_g = [col for lst in gr.chunks for (_, col) in lst]
        gr.col0 = min(cols_g) if cols_g else 0
        gr.npairs = len(cols_g)
        assert not cols_g or max(cols_g) - gr.col0 + 1 == gr.npairs
        gr.pairs = {}
        for wl in range(len(gr.windows)):
            gr.pairs[wl] = [(k, col) for k in range(gr.K)
                            for (wx, col) in gr.chunks[k] if wx == wl]

    # per-core arrays
    idx_all, dstv_all = [], []
    for c in range(NCORES):
        s_c, d_c, w_c, g_c, half = cores[c]
        slots = slot_of[c]
        idx_flat = np.zeros(S_total, dtype=np.int16)
        idx_flat[slots] = np.where(half == 0, s_c, s_c - HALF).astype(np.int16)
        idx_w = idx_flat.reshape(S_total // 16, 16).T
        idx_all.append(np.tile(idx_w, (8, 1)).copy())

        dstv = np.full((P, max(n_pairs, 1)), -1.0, dtype=np.float32)
        gr0 = np.array([groups[g].slot0 for g in g_c])
        k_loc = (slots - gr0) // P
        w_loc = w_c - g_c * M_WIN
        cols = np.array([pair_cols[key] for key in
                         zip(g_c.tolist(), k_loc.tolist(), w_loc.tolist())])
        dstv[slots % P, cols] = (d_c - w_c * P).astype(np.float32)
        dstv_all.append(dstv)

    return (dict(N=N, n_sh=n_sh, W=W, groups=groups, n_pairs=n_pairs,
                 S_total=S_total, dinv=dinv),
            idx_all, dstv_all)


# --------------------------------------------------------------------------
# bass kernel builder
# --------------------------------------------------------------------------

def build_nc(sched):
    N = sched["N"]
    n_sh = sched["n_sh"]
    W = sched["W"]
    groups = sched["groups"]
    n_pairs = sched["n_pairs"]
    S_total = sched["S_total"]
    n_groups = len(groups)
    maxsp = max((gr.npairs for gr in groups), default=1) * P
    wlast = n_sh - (W - 1) * P

    def wlen(w):
        return wlast if w == W - 1 else P

    # SWDGE ring capacity: calls consume nidx*packets/16 rows of the ring.
    for gr in groups:
        dpi = (ELEM * 2 + 16383) // 16384
        assert (max(gr.LA, gr.LB) * dpi) // 16 + 8 < 1024, (gr.LA, gr.LB)
    nc = bacc.Bacc(None, target_bir_lowering=False)

    xT = nc.dram_tensor("xT", [IN_C, n_sh], F32, kind="ExternalInput")
    idxs = nc.dram_tensor("idxs", [P, S_total // 16], I16, kind="ExternalInput")
    dstv_d = nc.dram_tensor("dstv", [P, max(n_pairs, 1)], BF16, kind="ExternalInput")
    dstvT_d = nc.dram_tensor("dstvT", [n_groups, maxsp], BF16, kind="ExternalInput")
    dinv_d = nc.dram_tensor("dinv_col", [P, W], F32, kind="ExternalInput")
    W1_d = nc.dram_tensor("W1", [IN_C, HID], F32, kind="ExternalInput")
    W2_d = nc.dram_tensor("W2", [HID, HID], F32, kind="ExternalInput")
    Wg1_d = nc.dram_tensor("Wg1", [HID, HEADS * HID], F32, kind="ExternalInput")
    Wg2_d = nc.dram_tensor("Wg2", [HEADS * HID, HEADS * OUT_C], F32, kind="ExternalInput")
    b1_d = nc.dram_tensor("b1r", [P, HID], F32, kind="ExternalInput")
    b2_d = nc.dram_tensor("b2r", [P, HID], F32, kind="ExternalInput")
    bg1_d = nc.dram_tensor("bg1r", [P, HEADS * HID], F32, kind="ExternalInput")
    bg2_d = nc.dram_tensor("bg2r", [P, OUT_C], F32, kind="ExternalInput")
    as1_d = nc.dram_tensor("as1r", [P, HEADS * HID], F32, kind="ExternalInput")
    ad1_d = nc.dram_tensor("ad1r", [P, HEADS * HID], F32, kind="ExternalInput")
    as2_d = nc.dram_tensor("as2r", [P, HEADS * OUT_C], F32, kind="ExternalInput")
    ad2_d = nc.dram_tensor("ad2r", [P, HEADS * OUT_C], F32, kind="ExternalInput")
    as1b_d = nc.dram_tensor("as1b", [P, HEADS * HID], BF16, kind="ExternalInput")
    as2b_d = nc.dram_tensor("as2b", [P, HEADS * OUT_C], BF16, kind="ExternalInput")
    iota_d = nc.dram_tensor("iota_row", [P, P], BF16, kind="ExternalInput")
    pcol_d = nc.dram_tensor("pcol", [P, 1], BF16, kind="ExternalInput")
    ident_d = nc.dram_tensor("ident", [P, P], F32, kind="ExternalInput")
    out_d = nc.dram_tensor("out", [n_sh, OUT_C], F32, kind="ExternalOutput")

    rg = [list(range(NCORES))]

    with tile.TileContext(nc) as tc:
        with (
            tc.tile_pool(name="const", bufs=1) as cpool,
            tc.tile_pool(name="gbuf", bufs=2) as gpool,
            tc.tile_pool(name="abuf", bufs=2) as apool,
            tc.tile_pool(name="bbuf", bufs=2) as bpool,
            tc.tile_pool(name="stbuf", bufs=2) as stpool,
            tc.tile_pool(name="work", bufs=3) as wpool,
            tc.tile_pool(name="small", bufs=4) as spool,
            tc.tile_pool(name="resid", bufs=1) as rpool,
            tc.tile_pool(name="uwin", bufs=4, space="PSUM") as upool,
            tc.tile_pool(name="admm", bufs=2, space="PSUM") as ampool,
            tc.tile_pool(name="epi", bufs=2, space="PSUM") as epool,
            tc.tile_pool(name="dram", bufs=1, space="DRAM") as dpool,
        ):
            nc.gpsimd.load_library(mlp)

            def load_const(src, shape, dt=F32):
                nm = f"c_{src.name}"
                t = cpool.tile(shape, dt, name=nm, tag=nm)
                nc.sync.dma_start(t[:], src[:])
                return t

            idx_sb = load_const(idxs, [P, S_total // 16], I16)
            dstv_sb = load_const(dstv_d, [P, max(n_pairs, 1)], BF16)
            dinv_sb = load_const(dinv_d, [P, W])
            W1_sb = load_const(W1_d, [IN_C, HID])
            W2_sb = load_const(W2_d, [HID, HID])
            Wg1_sb = load_const(Wg1_d, [HID, HEADS * HID])
            Wg2_sb = load_const(Wg2_d, [HEADS * HID, HEADS * OUT_C])
            b1_sb = load_const(b1_d, [P, HID])
            b2_sb = load_const(b2_d, [P, HID])
            bg1_sb = load_const(bg1_d, [P, HEADS * HID])
            bg2_sb = load_const(bg2_d, [P, OUT_C])
            att_sb = {3: (load_const(as1_d, [P, HEADS * HID]),
                          load_const(ad1_d, [P, HEADS * HID])),
                      4: (load_const(as2_d, [P, HEADS * OUT_C]),
                          load_const(ad2_d, [P, HEADS * OUT_C]))}
            attb_sb = {3: load_const(as1b_d, [P, HEADS * HID], BF16),
                       4: load_const(as2b_d, [P, HEADS * OUT_C], BF16)}
            iota_sb = load_const(iota_d, [P, P], BF16)
            pcol_sb = load_const(pcol_d, [P, 1], BF16)
            ident_sb = load_const(ident_d, [P, P])

            # per-GAT-layer residents
            ad_sh = {L: rpool.tile([P, 2 * W], F32, tag=f"adsh{L}", name=f"adsh{L}") for L in (3, 4)}
            adb_sh = {L: rpool.tile([P, 2 * W], BF16, tag=f"adbsh{L}", name=f"adbsh{L}") for L in (3, 4)}
            asw_sh = {L: rpool.tile([P, 2 * W], F32, tag=f"aswsh{L}", name=f"aswsh{L}") for L in (3, 4)}
            asmax = {L: rpool.tile([P, 2], F32, tag=f"asmax{L}", name=f"asmax{L}") for L in (3, 4)}
            admax = {L: rpool.tile([P, 2], F32, tag=f"admax{L}", name=f"admax{L}") for L in (3, 4)}
            negm = {L: rpool.tile([P, 2], F32, tag=f"negm{L}", name=f"negm{L}") for L in (3, 4)}

            ag_in = {L: dpool.tile([n_sh, ELEM], BF16, tag=f"agin{L}", name=f"agin{L}")
                     for L in (1, 2, 3, 4)}
            T = {L: dpool.tile([N, ELEM], BF16, tag=f"T{L}", name=f"Tbl{L}",
                               addr_space=("Local" if DBG_NO_SHARED else "Shared"))
                 for L in (1, 2, 3, 4)}
            mx_in = {L: dpool.tile([1, 8], F32, tag=f"mxin{L}", name=f"mxin{L}") for L in (3, 4)}
            mx_out = {L: dpool.tile([1, 8], F32, tag=f"mxout{L}", name=f"mxout{L}",
                                    addr_space="Shared") for L in (3, 4)}

            for L in (3, 4):
                nc.vector.memset(asmax[L][:], -3.0e38)
                nc.vector.memset(admax[L][:], -3.0e38)
                nc.vector.memset(ad_sh[L][:], 0.0)
                nc.vector.memset(asw_sh[L][:], 0.0)

            # ------------- helpers -------------
            def build_gat_row(L, w, h_ps):
                """h_ps [P, 2C] in PSUM -> bf16 table row + resident as/ad."""
                C = CDIM[L]
                as_sb, adr_sb = att_sb[L]
                n = wlen(w)
                row = wpool.tile([P, ELEM], BF16, tag="trow")
                nc.vector.tensor_copy(row[:, 0:2 * C], h_ps[:, 0:2 * C])
                prod = wpool.tile([P, HEADS * C], F32, tag="prod")
                red = spool.tile([P, 2], F32, tag="red")
                nc.vector.tensor_tensor(out=prod[:], in0=h_ps[:], in1=as_sb[:], op=OP.mult)
                nc.vector.tensor_reduce(
                    out=red[:],
                    in_=prod[:].rearrange("p (h c) -> p h c", h=HEADS),
                    axis=mybir.AxisListType.X, op=OP.add)
                nc.vector.tensor_copy(asw_sh[L][:, 2 * w:2 * w + 2], red[:])
                nc.vector.tensor_tensor(out=asmax[L][:], in0=asmax[L][:],
                                        in1=red[:], op=OP.max)
                red2 = spool.tile([P, 2], F32, tag="red2")
                nc.vector.tensor_tensor(out=prod[:], in0=h_ps[:], in1=adr_sb[:], op=OP.mult)
                nc.vector.tensor_reduce(
                    out=red2[:],
                    in_=prod[:].rearrange("p (h c) -> p h c", h=HEADS),
                    axis=mybir.AxisListType.X, op=OP.add)
                nc.vector.tensor_copy(ad_sh[L][:, 2 * w:2 * w + 2], red2[:])
                nc.vector.tensor_copy(adb_sh[L][:, 2 * w:2 * w + 2], red2[:])
                nc.vector.tensor_tensor(out=admax[L][:], in0=admax[L][:],
                                        in1=red2[:], op=OP.max)
                nc.sync.dma_start(ag_in[L][w * P:w * P + n, :], row[:n, :])

            def finish_gcn_table(layer, w, r_sb):
                """relu'd [P, HID] output of GCN layer -> next layer's table row."""
                n = wlen(w)
                rT_ps = epool.tile([P, P], F32, tag="epi")
                nc.tensor.transpose(out=rT_ps[:HID, :], in_=r_sb[:], identity=ident_sb[:])
                rT_sb = wpool.tile([HID, P], F32, tag="rT")
                nc.vector.tensor_copy(rT_sb[:], rT_ps[:HID, :])
                if layer == 1:
                    h_ps = epool.tile([P, P], F32, tag="epi")
                    nc.tensor.matmul(h_ps[:, :HID], lhsT=rT_sb[:], rhs=W2_sb[:],
                                     start=True, stop=True)
                    t_sb = wpool.tile([P, HID], BF16, tag="trow")
                    nc.vector.tensor_scalar(out=t_sb[:], in0=h_ps[:, :HID],
                                            scalar1=dinv_sb[:, w:w + 1],
                                            scalar2=None, op0=OP.mult)
                    nc.sync.dma_start(ag_in[2][w * P:w * P + n, 0:HID], t_sb[:n, :])
                else:
                    h_ps = epool.tile([P, P], F32, tag="epi")
                    nc.tensor.matmul(h_ps[:, :HEADS * HID], lhsT=rT_sb[:], rhs=Wg1_sb[:],
                                     start=True, stop=True)
                    build_gat_row(3, w, h_ps[:, :HEADS * HID])

            def epilogue(layer, w, U):
                n = wlen(w)
                C = CDIM[layer]
                val = VAL[layer]
                own = wpool.tile([P, val], BF16, tag="own")
                if n < P:
                    nc.vector.memset(own[:], 0.0)
                nc.sync.dma_start(own[:n, :], ag_in[layer][w * P:w * P + n, 0:val])
                own_f = wpool.tile([P, val], F32, tag="ownf")
                nc.vector.tensor_copy(own_f[:], own[:])

                if layer <= 2:
                    t = wpool.tile([P, HID], F32, tag="t1")
                    nc.vector.tensor_tensor(out=t[:], in0=U[:], in1=own_f[:], op=OP.add)
                    nc.vector.tensor_scalar(out=t[:], in0=t[:],
                                            scalar1=dinv_sb[:, w:w + 1],
                                            scalar2=None, op0=OP.mult)
                    nc.vector.tensor_tensor(out=t[:], in0=t[:],
                                            in1=(b1_sb if layer == 1 else b2_sb)[:],
                                            op=OP.add)
                    r = wpool.tile([P, HID], F32, tag="r1")
                    nc.scalar.activation(r[:], t[:], AF.Relu)
                    finish_gcn_table(layer, w, r)
                    return

                # GAT: add self-loop term, then normalize
                es = spool.tile([P, 2], F32, tag="es")
                nc.vector.tensor_tensor(out=es[:], in0=asw_sh[layer][:, 2 * w:2 * w + 2],
                                        in1=ad_sh[layer][:, 2 * w:2 * w + 2], op=OP.add)
                est = spool.tile([P, 2], F32, tag="est")
                nc.vector.tensor_scalar(out=est[:], in0=es[:], scalar1=NEG_SLOPE,
                                        scalar2=None, op0=OP.mult)
                nc.vector.tensor_tensor(out=es[:], in0=es[:], in1=est[:], op=OP.max)
                exs = spool.tile([P, 2], F32, tag="exs")
                for h in range(2):
                    nc.scalar.activation(exs[:, h:h + 1], es[:, h:h + 1], AF.Exp,
                                         bias=negm[layer][:, h:h + 1], scale=1.0)
                stg = wpool.tile([P, 2 * (C + 1)], F32, tag="stgs")
                for h in range(2):
                    nc.vector.tensor_scalar(
                        out=stg[:, h * (C + 1):h * (C + 1) + C],
                        in0=own_f[:, h * C:(h + 1) * C],
                        scalar1=exs[:, h:h + 1], scalar2=None, op0=OP.mult)
                    nc.vector.tensor_copy(stg[:, h * (C + 1) + C:(h + 1) * (C + 1)],
                                          exs[:, h:h + 1])
                nc.vector.tensor_tensor(out=U[:], in0=U[:], in1=stg[:], op=OP.add)

                sden = spool.tile([P, 2], F32, tag="sden")
                for h in range(2):
                    nc.vector.tensor_scalar(out=sden[:, h:h + 1],
                                            in0=U[:, h * (C + 1) + C:(h + 1) * (C + 1)],
                                            scalar1=1.0e-30, scalar2=None, op0=OP.max)
                rs = spool.tile([P, 2], F32, tag="rs")
                nc.vector.reciprocal(rs[:], sden[:])

                if layer == 3:
                    cat = wpool.tile([P, HEADS * HID], F32, tag="cat")
                    for h in range(2):
                        nc.vector.tensor_scalar(
                            out=cat[:, h * C:(h + 1) * C],
                            in0=U[:, h * (C + 1):h * (C + 1) + C],
                            scalar1=rs[:, h:h + 1], scalar2=None, op0=OP.mult)
                    nc.vector.tensor_tensor(out=cat[:], in0=cat[:], in1=bg1_sb[:], op=OP.add)
                    r = wpool.tile([P, HEADS * HID], F32, tag="cat2")
                    nc.scalar.activation(r[:], cat[:], AF.Relu)
                    rT_ps = epool.tile([P, P], F32, tag="epi")
                    nc.tensor.transpose(out=rT_ps[:], in_=r[:], identity=ident_sb[:])
                    rT_sb = wpool.tile([P, P], F32, tag="rT")
                    nc.vector.tensor_copy(rT_sb[:], rT_ps[:])
                    h_ps = epool.tile([P, P], F32, tag="epi")
                    nc.tensor.matmul(h_ps[:, :HEADS * OUT_C], lhsT=rT_sb[:], rhs=Wg2_sb[:],
                                     start=True, stop=True)
                    build_gat_row(4, w, h_ps[:, :HEADS * OUT_C])
                    return

                # layer 4: mean heads + bias + log_softmax -> output
                m0 = spool.tile([P, OUT_C], F32, tag="m0")
                nc.vector.tensor_scalar(out=m0[:], in0=U[:, 0:OUT_C],
                                        scalar1=rs[:, 0:1], scalar2=0.5,
                                        op0=OP.mult, op1=OP.mult)
                m1 = spool.tile([P, OUT_C], F32, tag="m1")
                nc.vector.tensor_scalar(out=m1[:], in0=U[:, OUT_C + 1:2 * OUT_C + 1],
                                        scalar1=rs[:, 1:2], scalar2=0.5,
                                        op0=OP.mult, op1=OP.mult)
                z = wpool.tile([P, OUT_C], F32, tag="z")
                nc.vector.tensor_tensor(out=z[:], in0=m0[:], in1=m1[:], op=OP.add)
                nc.vector.tensor_tensor(out=z[:], in0=z[:], in1=bg2_sb[:], op=OP.add)
                mx = spool.tile([P, 1], F32, tag="mx")
                nc.vector.tensor_reduce(out=mx[:], in_=z[:],
                                        axis=mybir.AxisListType.X, op=OP.max)
                nmx = spool.tile([P, 1], F32, tag="nmx")
                nc.vector.tensor_scalar(out=nmx[:], in0=mx[:], scalar1=-1.0,
                                        scalar2=None, op0=OP.mult)
                ez = wpool.tile([P, OUT_C], F32, tag="ez")
                ssum = spool.tile([P, 1], F32, tag="ssum")
                nc.scalar.activation(ez[:], z[:], AF.Exp, bias=nmx[:], scale=1.0,
                                     accum_out=ssum[:])
                lns = spool.tile([P, 1], F32, tag="lns")
                nc.scalar.activation(lns[:], ssum[:], AF.Ln)
                o = wpool.tile([P, OUT_C], F32, tag="o")
                nc.vector.tensor_scalar(out=o[:], in0=z[:], scalar1=mx[:],
                                        scalar2=lns[:], op0=OP.subtract, op1=OP.subtract)
                nc.sync.dma_start(out_d[w * P:w * P + n, :], o[:n, :])

            def edge_pass(layer):
                is_gat = layer >= 3
                C = CDIM[layer]
                ucols = HID if not is_gat else HEADS * (C + 1)

                if is_gat and DBG_NO_MAXRED:
                    nc.vector.memset(negm[layer][:], 0.0)
                elif is_gat:
                    nc.gpsimd.partition_all_reduce(
                        out_ap=asmax[layer][:], in_ap=asmax[layer][:], channels=P,
                        reduce_op=bass_isa.ReduceOp.max)
                    nc.gpsimd.partition_all_reduce(
                        out_ap=admax[layer][:], in_ap=admax[layer][:], channels=P,
                        reduce_op=bass_isa.ReduceOp.max)
                    mx8 = spool.tile([1, 8], F32, tag="mx8")
                    nc.vector.memset(mx8[:], -3.0e38)
                    nc.vector.tensor_copy(mx8[:, 0:2], asmax[layer][0:1, :])
                    nc.sync.dma_start(mx_in[layer][:], mx8[:])
                    if DBG_LOCAL_COLL:
                        nc.sync.dma_start(mx_out[layer][:], mx_in[layer][:])
                    else:
                        nc.gpsimd.collective_compute(
                            "AllReduce", OP.max, replica_groups=rg,
                            ins=[mx_in[layer].opt()], outs=[mx_out[layer].opt()])
                    asg = spool.tile([1, 8], F32, tag="asg8")
                    nc.sync.dma_start(asg[:], mx_out[layer][:])
                    asgb = spool.tile([P, 8], F32, tag="asgb")
                    nc.gpsimd.partition_broadcast(out_ap=asgb[:], in_ap=asg[:])
                    nc.vector.tensor_tensor(out=negm[layer][:], in0=asgb[:, 0:2],
                                            in1=admax[layer][:], op=OP.add)
                    nc.vector.tensor_scalar(out=negm[layer][:], in0=negm[layer][:],
                                            scalar1=-1.0, scalar2=None, op0=OP.mult)

                for g, gr in enumerate(groups):
                    K = gr.K
                    npg = gr.npairs
                    col0 = gr.col0
                    G = gpool.tile([P, K * ELEM], BF16, tag="G")
                    kA = gr.LA // P
                    if gr.LA:
                        creg = nc.gpsimd.alloc_register(f"gcA_{layer}_{g}")
                        mv = nc.gpsimd.reg_mov(creg, gr.LA)
                        gi = nc.gpsimd.dma_gather(
                            G[:, :kA * ELEM].rearrange("p (n e) -> p n e", e=ELEM),
                            T[layer][:],
                            idx_sb[:, gr.slot0 // 16:(gr.slot0 + gr.LA) // 16],
                            gr.LA, creg, ELEM,
                            single_packet=(gr.LA <= 1024))
                        add_dep_helper(gi.ins, mv.ins, sync=False,
                                       reason="gather count reg def before use")
                    if gr.LB:
                        creg = nc.gpsimd.alloc_register(f"gcB_{layer}_{g}")
                        mv = nc.gpsimd.reg_mov(creg, gr.LB)
                        gi = nc.gpsimd.dma_gather(
                            G[:, kA * ELEM:].rearrange("p (n e) -> p n e", e=ELEM),
                            T[layer][HALF:, :],
                            idx_sb[:, (gr.slot0 + gr.LA) // 16:
                                   (gr.slot0 + gr.LA + gr.LB) // 16],
                            gr.LB, creg, ELEM,
                            single_packet=(gr.LB <= 1024))
                        add_dep_helper(gi.ins, mv.ins, sync=False,
                                       reason="gather count reg def before use")

                    # batched one-hot lhsT build: all of the group's pairs at once
                    A_big = apool.tile([P, npg * P], BF16, tag="Abig")
                    nc.vector.tensor_tensor(
                        out=A_big[:].rearrange("p (m q) -> p m q", q=P),
                        in0=dstv_sb[:, col0:col0 + npg][:, :, None].to_broadcast(
                            [P, npg, P]),
                        in1=iota_sb[:, 0:P][:, None, :].to_broadcast([P, npg, P]),
                        op=OP.is_equal)

                    G3 = G[:].rearrange("p (k e) -> p k e", e=ELEM)
                    if is_gat:
                        # B matrices (transposed one-hots) built in place on Pool
                        dTb = bpool.tile([P, npg * P], BF16, tag="Bbig")
                        nc.sync.dma_start(dTb[0:1, 0:npg * P],
                                          dstvT_d[g:g + 1, 0:npg * P])
                        nc.gpsimd.partition_broadcast(out_ap=dTb[:, 0:npg * P],
                                                      in_ap=dTb[0:1, 0:npg * P])
                        nc.vector.tensor_tensor(
                            out=dTb[:].rearrange("p (m q) -> p m q", q=P),
                            in0=dTb[:].rearrange("p (m q) -> p m q", q=P),
                            in1=pcol_sb[:, 0:1][:, :, None].to_broadcast(
                                [P, npg, P]),
                            op=OP.is_equal)

                        # ad[dst] per edge chunk via per-pair matmuls
                        admm = ampool.tile([P, 2 * K], F32, tag="admm",
                                           name=f"admm{layer}_{g}")
                        for k in range(K):
                            plist = gr.chunks[k]
                            for pi, (wl, col) in enumerate(plist):
                                w_abs = gr.windows[wl]
                                nc.tensor.matmul(
                                    admm[:, 2 * k:2 * k + 2],
                                    lhsT=dTb[:, (col - col0) * P:(col - col0 + 1) * P],
                                    rhs=adb_sh[layer][:, 2 * w_abs:2 * w_abs + 2],
                                    start=(pi == 0), stop=(pi == len(plist) - 1))

                        # as[src] per edge reconstructed from gathered h
                        asb = attb_sb[layer]
                        prodt = stpool.tile([P, K * 2 * C], BF16, tag="prodt")
                        nc.vector.tensor_tensor(
                            out=prodt[:].rearrange("p (k h c) -> p k h c",
                                                   h=2, c=C),
                            in0=G3[:, :, 0:2 * C].rearrange(
                                "p k (h c) -> p k h c", c=C),
                            in1=asb[:, None, :].to_broadcast(
                                [P, K, 2 * C]).rearrange(
                                "p k (h c) -> p k h c", c=C),
                            op=OP.mult)
                        asg_b = wpool.tile([P, 2 * K], F32, tag="asgbig")
                        nc.vector.tensor_reduce(
                            out=asg_b[:].rearrange("p (k h) -> p k h", h=2),
                            in_=prodt[:].rearrange("p (k h c) -> p k h c",
                                                   h=2, c=C),
                            axis=mybir.AxisListType.X, op=OP.add)

                        # batched leaky-relu + exp pipeline
                        e_b = wpool.tile([P, 2 * K], F32, tag="ebig")
                        nc.vector.tensor_tensor(out=e_b[:], in0=admm[:],
                                                in1=asg_b[:], op=OP.add)
                        elk = wpool.tile([P, 2 * K], F32, tag="elkbig")
                        nc.vector.tensor_scalar(out=elk[:], in0=e_b[:],
                                                scalar1=NEG_SLOPE,
                                                scalar2=None, op0=OP.mult)
                        nc.vector.tensor_tensor(out=e_b[:], in0=e_b[:],
                                                in1=elk[:], op=OP.max)
                        nc.vector.tensor_tensor(
                            out=e_b[:].rearrange("p (k h) -> p k h", h=2),
                            in0=e_b[:].rearrange("p (k h) -> p k h", h=2),
                            in1=negm[layer][:, None, :].to_broadcast([P, K, 2]),
                            op=OP.add)
                        exs_b = wpool.tile([P, 2 * K], BF16, tag="exsbig")
                        nc.scalar.activation(exs_b[:], e_b[:], AF.Exp)

                        # staged rhs: [h*exs | exs] per head, bf16
                        stg = stpool.tile([P, K * 2 * (C + 1)], BF16, tag="stg")
                        st4 = stg[:].rearrange("p (k h c) -> p k h c",
                                               h=2, c=C + 1)
                        nc.vector.tensor_tensor(
                            out=st4[:, :, :, 0:C],
                            in0=G3[:, :, 0:2 * C].rearrange(
                                "p k (h c) -> p k h c", c=C),
                            in1=exs_b[:].rearrange(
                                "p (k h) -> p k h", h=2)[:, :, :, None
                                ].to_broadcast([P, K, 2, C]),
                            op=OP.mult)
                        nc.vector.tensor_copy(
                            st4[:, :, :, C:C + 1],
                            exs_b[:].rearrange("p (k h) -> p k h", h=2)[
                                :, :, :, None])

                    U = {wl: upool.tile([P, ucols], F32, tag="uwin",
                                        name=f"U{layer}_{g}_{wl}")
                         for wl in range(len(gr.windows))}
                    seen = {wl: 0 for wl in U}
                    total = {wl: len(gr.pairs[wl]) for wl in U}

                    for k in range(K):
                        plist = gr.chunks[k]
                        if not plist:
                            continue
                        if is_gat:
                            rhs = stg[:, k * 2 * (C + 1):(k + 1) * 2 * (C + 1)]
                        else:
                            rhs = G[:, k * ELEM:k * ELEM + HID]
                        for (wl, col) in plist:
                            nc.tensor.matmul(
                                U[wl][:],
                                lhsT=A_big[:, (col - col0) * P:(col - col0 + 1) * P],
                                rhs=rhs,
                                start=(seen[wl] == 0),
                                stop=(seen[wl] == total[wl] - 1))
                            seen[wl] += 1

                    for wl, w in enumerate(gr.windows):
                        epilogue(layer, w, U[wl])

            def allgather(L):
                if DBG_LOCAL_COLL:
                    for c in range(NCORES):
                        nc.sync.dma_start(T[L][c * n_sh:(c + 1) * n_sh, :],
                                          ag_in[L][:])
                    return
                nc.gpsimd.collective_compute(
                    "AllGather", OP.bypass, replica_groups=rg,
                    ins=[ag_in[L].opt()], outs=[T[L].opt()])

            # ---------------- main program ----------------
            for w in range(W):
                n = wlen(w)
                xw = wpool.tile([IN_C, P], F32, tag="xw")
                nc.sync.dma_start(xw[:, :n], xT[:, w * P:w * P + n])
                h_ps = epool.tile([P, P], F32, tag="epi")
                nc.tensor.matmul(h_ps[:n, :HID], lhsT=xw[:, :n],
                                 rhs=W1_sb[:], start=True, stop=True)
                t_sb = wpool.tile([P, HID], BF16, tag="trow")
                nc.vector.tensor_scalar(out=t_sb[:n, :], in0=h_ps[:n, :HID],
                                        scalar1=dinv_sb[:n, w:w + 1],
                                        scalar2=None, op0=OP.mult)
                nc.sync.dma_start(ag_in[1][w * P:w * P + n, 0:HID], t_sb[:n, :])

            allgather(1)
            if DBG_MAX_LAYER >= 1:
                edge_pass(1)
            if DBG_MAX_LAYER >= 2:
                allgather(2)
                edge_pass(2)
            if DBG_MAX_LAYER >= 3:
                allgather(3)
                edge_pass(3)
            if DBG_MAX_LAYER >= 4:
                allgather(4)
                edge_pass(4)
            if DBG_MAX_LAYER < 4:
                zt = wpool.tile([P, OUT_C], F32, tag="o")
                nc.vector.memset(zt[:], 0.0)
                for w in range(W):
                    nc.sync.dma_start(out_d[w * P:w * P + wlen(w), :], zt[:wlen(w), :])

    return nc


# --------------------------------------------------------------------------
# host entry
# --------------------------------------------------------------------------

def make_inmaps(x, sched, idx_all, dstv_all, weights):
    (W1, b1, W2, b2, Wg1, as1, ad1, bg1, Wg2, as2, ad2, bg2) = weights
    n_sh = sched["n_sh"]
    Wn = sched["W"]
    dinv = sched["dinv"]
    groups = sched["groups"]
    n_groups = len(groups)
    maxsp = max((gr.npairs for gr in groups), default=1) * P

    def rep(v, dt=np.float32):
        v = np.asarray(v, np.float32).reshape(1, -1)
        return np.tile(v, (P, 1)).astype(dt).copy()

    iota = np.tile(np.arange(P, dtype=np.float32)[None, :], (P, 1)).astype(BFNP)
    pcol = np.arange(P, dtype=np.float32)[:, None].astype(BFNP)
    ident = np.eye(P, dtype=np.float32)

    in_maps = []
    for c in range(NCORES):
        flat = np.zeros(Wn * P, np.float32)
        flat[:n_sh] = dinv[c * n_sh:(c + 1) * n_sh]
        dstv = dstv_all[c]
        dstvT = np.full((n_groups, maxsp), -1.0, np.float32)
        for g, gr in enumerate(groups):
            if gr.npairs:
                blk = dstv[:, gr.col0:gr.col0 + gr.npairs].T.reshape(-1)
                dstvT[g, :gr.npairs * P] = blk
        in_maps.append({
            "xT": np.ascontiguousarray(
                np.asarray(x)[c * n_sh:(c + 1) * n_sh].T).astype(np.float32),
            "idxs": idx_all[c],
            "dstv": dstv.astype(BFNP),
            "dstvT": dstvT.astype(BFNP),
            "dinv_col": np.ascontiguousarray(flat.reshape(Wn, P).T),
            "W1": np.asarray(W1, np.float32), "W2": np.asarray(W2, np.float32),
            "Wg1": np.asarray(Wg1, np.float32), "Wg2": np.asarray(Wg2, np.float32),
            "b1r": rep(b1), "b2r": rep(b2), "bg1r": rep(bg1), "bg2r": rep(bg2),
            "as1r": rep(np.asarray(as1).reshape(-1)),
            "ad1r": rep(np.asarray(ad1).reshape(-1)),
            "as2r": rep(np.asarray(as2).reshape(-1)),
            "ad2r": rep(np.asarray(ad2).reshape(-1)),
            "as1b": rep(np.asarray(as1).reshape(-1), BFNP),
            "as2b": rep(np.asarray(as2).reshape(-1), BFNP),
            "iota_row": iota, "pcol": pcol, "ident": ident,
        })
    return in_maps


def _run_bench(inputs, n_iters=6):
    """Compile once, then time repeated executions with device-resident
    inputs (fresh donated zero-outputs each iteration). Returns
    (out, best_seconds)."""
    import time
    import jax
    from jax.sharding import Mesh, PartitionSpec, NamedSharding
    from jax.experimental.shard_map import shard_map
    from concourse import bass2jax as b2j

    x = np.asarray(inputs["x"])
    N = x.shape[0]
    sched, idx_all, dstv_all = build_schedule(np.asarray(inputs["edge_index"]), N)
    weights = tuple(inputs[k] for k in
                    ("W1", "b1", "W2", "b2", "Wg1", "as1", "ad1", "bg1",
                     "Wg2", "as2", "ad2", "bg2"))
    in_maps = make_inmaps(x, sched, idx_all, dstv_all, weights)
    nc = build_nc(sched)
    nc.compile()

    b2j.install_neuronx_cc_hook()
    partition_name = nc.partition_id_tensor.name if nc.partition_id_tensor else None
    in_names, out_names, out_avals, zero_outs = [], [], [], []
    import concourse.mybir as mb
    for alloc in nc.m.functions[0].allocations:
        if not isinstance(alloc, mb.MemoryLocationSet):
            continue
        name = alloc.memorylocations[0].name
        if alloc.kind == "ExternalInput":
            if name != partition_name:
                in_names.append(name)
        elif alloc.kind == "ExternalOutput":
            out_names.append(name)
            shape = tuple(alloc.tensor_shape)
            dtype = mb.dt.np(alloc.dtype)
            out_avals.append(jax.core.ShapedArray(shape, dtype))
            zero_outs.append(np.zeros(shape, dtype))
    n_params = len(in_names)
    n_outs = len(out_avals)
    in_names_full = in_names + out_names + ([partition_name] if partition_name else [])

    def _body(*args):
        operands = list(args)
        if partition_name is not None:
            operands.append(b2j.partition_id_tensor())
        return tuple(b2j._bass_exec_p.bind(
            *operands, out_avals=tuple(out_avals), in_names=tuple(in_names_full),
            out_names=tuple(out_names), lowering_input_output_aliases=(),
            sim_require_finite=True, sim_require_nnan=True, nc=nc))

    donate = tuple(range(n_params, n_params + n_outs))
    devices = jax.devices()[:NCORES]
    mesh = Mesh(np.asarray(devices), ("core",))
    PS = PartitionSpec("core")
    sharded = jax.jit(
        shard_map(_body, mesh=mesh, in_specs=(PS,) * (n_params + n_outs),
                  out_specs=(PS,) * n_outs, check_rep=False),
        donate_argnums=donate, keep_unused=True)
    shd = NamedSharding(mesh, PS)
    concat_in = [
        jax.device_put(np.concatenate([np.asarray(in_maps[c][nm]) for c in range(NCORES)],
                                      axis=0), shd)
        for nm in in_names]
    zshapes = [(NCORES * z.shape[0], *z.shape[1:]) for z in zero_outs]

    best = None
    outs_np = None
    for it in range(n_iters):
        zs = [jax.device_put(np.zeros(s, np.float32), shd) for s in zshapes]
        jax.block_until_ready(zs)
        t0 = time.perf_counter()
        outs = sharded(*concat_in, *zs)
        jax.block_until_ready(outs)
        dt = time.perf_counter() - t0
        print(f"  iter {it}: {dt*1e3:.3f} ms")
        if best is None or dt < best:
            best = dt
        outs_np = [np.asarray(o) for o in outs]
    i_out = out_names.index("out")
    out = outs_np[i_out].reshape(NCORES, -1, OUT_C).reshape(-1, OUT_C)
    return out.astype(np.float32), best


def _run(inputs, trace=False, tmpdir=None):
    x = np.asarray(inputs["x"])
    N = x.shape[0]
    sched, idx_all, dstv_all = build_schedule(np.asarray(inputs["edge_index"]), N)
    weights = tuple(inputs[k] for k in
                    ("W1", "b1", "W2", "b2", "Wg1", "as1", "ad1", "bg1",
                     "Wg2", "as2", "ad2", "bg2"))
    in_maps = make_inmaps(x, sched, idx_all, dstv_all, weights)
    nc = build_nc(sched)
    nc.compile()
    res = run_bass_kernel_spmd(nc, in_maps, list(range(NCORES)),
                               trace=trace, tmpdir=tmpdir)
    out = np.concatenate([res.results[c]["out"] for c in range(NCORES)], axis=0)
    return out.astype(np.float32), res


def kernel(x, edge_index, W1, b1, W2, b2, Wg1, as1, ad1, bg1, Wg2, as2, ad2, bg2):
    out, _ = _run(dict(x=x, edge_index=edge_index, W1=W1, b1=b1, W2=W2, b2=b2,
                       Wg1=Wg1, as1=as1, ad1=ad1, bg1=bg1, Wg2=Wg2, as2=as2,
                       ad2=ad2, bg2=bg2))
    return out
